# revision 2
# baseline (speedup 1.0000x reference)
"""Fused self-attention (FCSelfAttention) Trainium2 Bass kernel.

Problem: X:[4,2048,512] fp32, W_qkv:[512,1536], W_out:[512,512], b_out:[512]
  qkv = X @ W_qkv ; q,k,v -> heads (B,H=8,N=2048,DH=64)
  scores[n,m] = k_n . q_m * DH**-0.5 ; softmax over m (query axis)
  out[n] = sum_e att[n,e] v[e] ; merge heads ; @ W_out + b_out

Sharding (8 cores): batch x head-group. Core c handles batch b=c//2 and
heads 4g..4g+3 where g=c%2 (data parallel over B=4, tensor parallel over
H=8 in halves). Each core computes a partial output projection for its
batch; the host sums the two partials per batch and adds b_out.

Device algorithm (per core), flash-style with scores kept transposed so
the softmax axis lands on the TensorE contraction axis:
  S^T[m,n] = sum_d QT[d,m] KT[d,n]        (m = softmax axis, on partitions)
  P^T = exp(S^T * SCALE)                   (no max subtraction; |S| < 9)
  PV:  lhsT = V_aug[e, 0:65] (col 64 = ones) -> psum[0:64]=out^T, psum[64]=Z

v2 changes over the first working kernel:
  - software pipelining: the score matmuls for block g+1 are emitted
    BEFORE block g's PV matmuls, so the (in-order) PE always has the
    next exp's input ready early and ACT never waits on PE backlog.
  - two of every 16 m-blocks compute exp on the DVE instead of ACT via
    the integer exp trick (bits of bf16 = s*16*log2e + 16250, computed
    by one tensor_scalar with int16 output); this offloads ~12.5% of
    the exp stream off the pacing Scalar engine at ~0.4% output error.
  - staged startup: only the tensors needed by quarter 0 gate the first
    exp; everything else (xt tail, remaining V/projection pieces) is
    drained through the per-block pending queue.
  - no DMAs ride the Scalar queue (they'd serialize with ACTIVATEs).
"""

import sys

import numpy as np

_B, _N, _DIM = 4, 2048, 512
_H, _DH = 8, 64
_SCALE = _DH ** -0.5
_NCORES = 8
_HPC = 4              # heads per core
_HL = _HPC * _DH      # 256 local inner dim
_TC = _N // 128       # 16 token chunks
_KC = _DIM // 128     # 4 contraction chunks for projections

# DVE integer-exp constants: bf16 bits of exp(s*SCALE) ~= s*A + B
_EXPA = 128.0 * np.log2(np.e) * _SCALE      # 16*log2(e)
_EXPB = 127.0 * 128.0 - 6.0                  # bias, c=-6 tuned offline
_DVE_ECS = (5, 11)                           # m-blocks per quarter on DVE

_cache = {}


def _emit(tc, xt, wq, wk, wv, wo, out, mybir):
    nc = tc.nc
    dt = mybir.dt
    f32, bf16, i16 = dt.float32, dt.bfloat16, dt.int16
    Exp = mybir.ActivationFunctionType.Exp
    Copy = mybir.ActivationFunctionType.Copy
    Alu = mybir.AluOpType

    from contextlib import ExitStack

    with ExitStack() as ctx:
        weights = ctx.enter_context(tc.tile_pool(name="weights", bufs=1))
        xtp = ctx.enter_context(tc.tile_pool(name="xtp", bufs=1))
        qkp = ctx.enter_context(tc.tile_pool(name="qkp", bufs=1))
        vap = ctx.enter_context(tc.tile_pool(name="vap", bufs=1))
        atp = ctx.enter_context(tc.tile_pool(name="atp", bufs=1))
        ptp = ctx.enter_context(tc.tile_pool(name="ptp", bufs=4))
        zp = ctx.enter_context(tc.tile_pool(name="zp", bufs=2))
        zdp = ctx.enter_context(tc.tile_pool(name="zdp", bufs=2, space="DRAM"))
        outp = ctx.enter_context(tc.tile_pool(name="outp", bufs=1))
        psA = ctx.enter_context(tc.tile_pool(name="psA", bufs=2, space="PSUM"))
        psOp = ctx.enter_context(tc.tile_pool(name="psO", bufs=2, space="PSUM"))
        psB = ctx.enter_context(tc.tile_pool(name="psB", bufs=2, space="PSUM"))

        # ---- input DMAs, staged ------------------------------------------
        # Stage A gates the first exp: xt token-cols 0:512 (quarter 0 of
        # m and n), all projection weights.  Stages B/C (xt 512:1024,
        # 1024:2048) land while quarter 0 runs.  Nothing rides nc.scalar.
        xt_sb = []
        for kc in range(_KC):
            t = xtp.tile([128, _N], bf16, tag=f"xt{kc}", name=f"xt{kc}")
            xt_sb.append(t)
        for kc in range(_KC):
            (nc.sync if kc % 2 else nc.gpsimd).dma_start(
                xt_sb[kc][:, 0:512], xt[kc * 128:(kc + 1) * 128, 0:512])

        wq_sb, wk_sb, wv_sb = [], [], []
        for name, dram, lst in (("wq", wq, wq_sb), ("wk", wk, wk_sb),
                                ("wv", wv, wv_sb)):
            for kc in range(_KC):
                t = weights.tile([128, _HL], bf16, tag=f"{name}{kc}",
                                 name=f"{name}{kc}")
                nc.gpsimd.dma_start(t, dram[kc * 128:(kc + 1) * 128, :])
                lst.append(t)
        wo_sb = []
        for pair in range(2):
            t = weights.tile([128, _DIM], bf16, tag=f"wo{pair}",
                             name=f"wo{pair}")
            nc.gpsimd.dma_start(t, wo[pair * 128:(pair + 1) * 128, :])
            wo_sb.append(t)
        ones11 = weights.tile([1, 1], f32, tag="ones11", name="ones11")
        nc.vector.memset(ones11, 1.0)

        # stages B/C
        for lo, hi in ((512, 1024), (1024, _N)):
            for kc in range(_KC):
                (nc.sync if kc % 2 else nc.gpsimd).dma_start(
                    xt_sb[kc][:, lo:hi], xt[kc * 128:(kc + 1) * 128, lo:hi])

        # Warm the PE HAM clock with dummy matmuls while input DMAs land.
        dummy = xtp.tile([128, 512], bf16, tag="dummy", name="dummy")
        nc.vector.memset(dummy, 0.0)
        psw = psA.tile([128, 512], f32, tag="mm")
        for _ in range(8):
            nc.tensor.matmul(psw, lhsT=dummy[:, 0:128], rhs=dummy,
                             start=True, stop=True)

        # ---- qkv projections --------------------------------------------
        qt_sb = [None, None]
        kt_sb = [None, None]

        def project_qk_piece(name, wsb, lst, hc, tp, pool=None, tag="mm"):
            if lst[hc] is None:
                lst[hc] = qkp.tile([128, _N], bf16, tag=f"{name}{hc}",
                                   name=f"{name}{hc}")
            dst = lst[hc]
            ps = (pool or psA).tile([128, 512], f32, tag=tag)
            for kc in range(_KC):
                nc.tensor.matmul(
                    ps,
                    lhsT=wsb[kc][:, hc * 128:(hc + 1) * 128],
                    rhs=xt_sb[kc][:, tp * 512:(tp + 1) * 512],
                    start=(kc == 0), stop=(kc == _KC - 1),
                )
            nc.vector.tensor_copy(dst[:, tp * 512:(tp + 1) * 512], ps)

        # V augmented with a ones column: va[t][:, h, 0:64] = V, [..., 64]=1
        va_sb = []
        for t in range(_TC):
            va_sb.append(vap.tile([128, _HPC, 65], bf16, tag=f"va{t}",
                                  name=f"va{t}"))

        def v_piece(t, pool=None, tag="mm"):
            va = va_sb[t]
            nc.gpsimd.memset(va[:, :, 64:65], 1.0)
            ps = (pool or psA).tile([128, _HL], f32, tag=tag)
            for kc in range(_KC):
                nc.tensor.matmul(
                    ps,
                    lhsT=xt_sb[kc][:, t * 128:(t + 1) * 128],
                    rhs=wv_sb[kc],
                    start=(kc == 0), stop=(kc == _KC - 1),
                )
            nc.vector.tensor_copy(
                va[:, :, 0:64], ps.rearrange("p (h d) -> p h d", h=_HPC))

        # Minimum serial prefix: qt pieces 0-1, kt piece 0, va 0-3 (all
        # inside xt cols 0:512).  The rest drains through pending pops.
        project_qk_piece("qt", wq_sb, qt_sb, 0, 0, pool=psB, tag="mo")
        project_qk_piece("qt", wq_sb, qt_sb, 0, 1, pool=psB, tag="mo")
        project_qk_piece("kt", wk_sb, kt_sb, 0, 0, pool=psB, tag="mo")
        for t in range(4):
            v_piece(t, pool=psB, tag="mo")

        def mkv(t):
            return lambda: v_piece(t, pool=psB, tag="mo")

        def mkp(name, wsb, lst, hc, tp):
            return lambda: project_qk_piece(name, wsb, lst, hc, tp,
                                            pool=psB, tag="mo")

        # ---- attention state --------------------------------------------
        acc = []
        for t in range(_TC):
            acc.append(outp.tile([128, _DIM], f32, tag=f"acc{t}",
                                 name=f"acc{t}"))
        at_sb = [None, None]        # per pair, [128, N] (head rows stacked)
        zrec = [None] * _HPC

        def outproj_chunk(pair, t, store, wide=False):
            h0, h1 = 2 * pair, 2 * pair + 1
            tsl = slice(t * 128, (t + 1) * 128)
            ps0 = psB.tile([128, _DIM], f32, tag="mo")
            ps1 = (psOp if wide else psB).tile(
                [128, _DIM], f32, tag="po" if wide else "mo")
            nc.tensor.matmul(ps0, lhsT=at_sb[pair][0:64, tsl],
                             rhs=wo_sb[pair][0:64, :], start=True, stop=True)
            nc.tensor.matmul(ps1, lhsT=at_sb[pair][64:128, tsl],
                             rhs=wo_sb[pair][64:128, :], start=True, stop=True)
            if h0 == 0:
                nc.vector.tensor_scalar_mul(acc[t], ps0, zrec[h0][:, t:t + 1])
            else:
                nc.vector.scalar_tensor_tensor(
                    out=acc[t], in0=ps0, scalar=zrec[h0][:, t:t + 1],
                    in1=acc[t], op0=Alu.mult, op1=Alu.add,
                )
            nc.vector.scalar_tensor_tensor(
                out=acc[t], in0=ps1, scalar=zrec[h1][:, t:t + 1],
                in1=acc[t], op0=Alu.mult, op1=Alu.add,
            )
            if store:
                (nc.gpsimd if t % 2 else nc.sync).dma_start(
                    out[tsl, :], acc[t])

        def mkop(pair, t, store, wide=False):
            return lambda: outproj_chunk(pair, t, store, wide)

        def mkz(h, zr_row, q, qs, eng):
            def zchain():
                zd = zdp.tile([1, 512], f32, tag=f"zd{h % 2}")
                eng.dma_start(zd, zr_row[0:1, qs])
                zcol = zp.tile([128, 4], f32, tag=f"zcol{h % 2}")
                eng.dma_start(
                    zcol, zd.rearrange("o (j p) -> (o p) j", p=128))
                nc.vector.reciprocal(zrec[h][:, q * 4:(q + 1) * 4], zcol)
            return zchain

        def mkz_pe(h, zr_row, q):
            def zchain():
                pz = psB.tile([128, 4], f32, tag="mo")
                for j in range(4):
                    jj = q * 4 + j
                    nc.tensor.transpose(
                        pz[:, j:j + 1],
                        zr_row[0:1, jj * 128:(jj + 1) * 128], ones11)
                nc.vector.reciprocal(zrec[h][:, q * 4:(q + 1) * 4], pz)
            return zchain

        # ---- per-quarter pending-op schedules ---------------------------
        # quarter key (pair, q); each entry is a list of thunks popped one
        # per ec-block inside that quarter.
        nop = lambda: None  # noqa: E731
        sched = {}
        # pair0 q0: V ramp + remaining qt/kt pieces for pair 0.
        sched[(0, 0)] = [mkv(4), mkv(5), mkp("qt", wq_sb, qt_sb, 0, 2),
                         mkv(6), mkv(7), mkp("qt", wq_sb, qt_sb, 0, 3),
                         mkv(8), mkv(9), mkv(10),
                         mkp("kt", wk_sb, kt_sb, 0, 1),
                         mkv(11), mkv(12), mkv(13), mkv(14), mkv(15)]
        # later quarters get z-chains + outproj of the previous quarter
        # prepended at drain time; static part here: kt pieces for pair 0,
        # then pair 1's eight projection pieces spread over q1..q3.
        sched[(0, 1)] = [mkp("kt", wk_sb, kt_sb, 0, 2),
                         mkp("kt", wk_sb, kt_sb, 1, 0),
                         mkp("qt", wq_sb, qt_sb, 1, 0),
                         mkp("qt", wq_sb, qt_sb, 1, 1)]
        sched[(0, 2)] = [mkp("kt", wk_sb, kt_sb, 0, 3),
                         mkp("qt", wq_sb, qt_sb, 1, 2),
                         mkp("qt", wq_sb, qt_sb, 1, 3),
                         mkp("kt", wk_sb, kt_sb, 1, 1)]
        sched[(0, 3)] = [mkp("kt", wk_sb, kt_sb, 1, 2)]
        sched[(1, 0)] = [mkp("kt", wk_sb, kt_sb, 1, 3)]
        sched[(1, 1)] = []
        sched[(1, 2)] = []
        sched[(1, 3)] = []

        # ---- main loop: software-pipelined blocks ------------------------
        blocks = [(p, q, ec) for p in range(2) for q in range(4)
                  for ec in range(_TC)]

        for p in range(2):
            at_sb[p] = atp.tile([128, _N], bf16, tag=f"at{p}", name=f"at{p}")
            for h in (2 * p, 2 * p + 1):
                zrec[h] = zp.tile([128, _TC], f32, tag=f"zrec{h}",
                                  name=f"zrec{h}", bufs=1)
        zrow = {}
        for p in range(2):
            for h in range(2):
                zrow[(p, h)] = zp.tile([1, _N], f32, tag=f"zrow{p}{h}",
                                       name=f"zrow{p}{h}", bufs=1)

        def emit_scores(blk, ps):
            p, q, ec = blk
            ncol = q * 512
            nc.tensor.matmul(
                ps[:, 0:512],
                lhsT=qt_sb[p][0:64, ec * 128:(ec + 1) * 128],
                rhs=kt_sb[p][0:64, ncol:ncol + 512],
                start=True, stop=True,
            )
            nc.tensor.matmul(
                ps[:, 512:1024],
                lhsT=qt_sb[p][64:128, ec * 128:(ec + 1) * 128],
                rhs=kt_sb[p][64:128, ncol:ncol + 512],
                start=True, stop=True,
            )

        # scores for block 0 are part of the prefix
        ps_cur = psA.tile([128, 1024], f32, tag="mm")
        emit_scores(blocks[0], ps_cur)

        po0 = po1 = None
        pending = []
        for g, blk in enumerate(blocks):
            p, q, ec = blk
            h0, h1 = 2 * p, 2 * p + 1
            if ec == 0:
                po0 = psOp.tile([65, 512], f32, tag="po")
                po1 = psOp.tile([65, 512], f32, tag="po")
                pending = sched[(p, q)] + pending

            # 1) scores for the NEXT block (PE clears ACT's next dep early)
            if g + 1 < len(blocks):
                ps_next = psA.tile([128, 1024], f32, tag="mm")
                emit_scores(blocks[g + 1], ps_next)
            # 2) exp of the current block (ACT, or DVE integer-exp)
            pt = ptp.tile([128, 1024], bf16, tag="pt")
            if ec in _DVE_ECS:
                nc.vector.tensor_scalar(
                    out=pt.bitcast(i16), in0=ps_cur,
                    scalar1=float(_EXPA), scalar2=float(_EXPB),
                    op0=Alu.mult, op1=Alu.add,
                )
            else:
                nc.scalar.activation(pt, ps_cur, Exp, scale=_SCALE)
            ps_cur = ps_next
            # 3) PV accumulation
            nc.tensor.matmul(
                po0[0:65, :], lhsT=va_sb[ec][:, h0, :], rhs=pt[:, 0:512],
                start=(ec == 0), stop=(ec == _TC - 1),
            )
            nc.tensor.matmul(
                po1[0:65, :], lhsT=va_sb[ec][:, h1, :], rhs=pt[:, 512:1024],
                start=(ec == 0), stop=(ec == _TC - 1),
            )
            # 4) one deferred op
            if pending:
                pending.pop(0)()

            if ec == _TC - 1:
                # quarter drain: out^T rows -> at_sb, Z rows -> zrow
                qs = slice(q * 512, (q + 1) * 512)
                last_q = (p == 1 and q == 3)
                if last_q:
                    nc.vector.tensor_copy(zrow[(p, 0)][:, qs], po0[64:65, :])
                    nc.scalar.activation(at_sb[p][0:64, qs], po0[0:64, :],
                                         Copy)
                    nc.vector.tensor_copy(zrow[(p, 1)][:, qs], po1[64:65, :])
                    nc.scalar.activation(at_sb[p][64:128, qs], po1[0:64, :],
                                         Copy)
                else:
                    nc.vector.tensor_copy(at_sb[p][0:64, qs], po0[0:64, :])
                    nc.vector.tensor_copy(zrow[(p, 0)][:, qs], po0[64:65, :])
                    nc.vector.tensor_copy(at_sb[p][64:128, qs], po1[0:64, :])
                    nc.vector.tensor_copy(zrow[(p, 1)][:, qs], po1[64:65, :])

                if last_q:
                    # tail: z via PE transposes (ACT/DVE are free now)
                    mkz_pe(h0, zrow[(p, 0)], q)()
                    mkz_pe(h1, zrow[(p, 1)], q)()
                    for t in range(q * 4, (q + 1) * 4):
                        outproj_chunk(p, t, store=True, wide=True)
                else:
                    nxt = [mkz(h0, zrow[(p, 0)], q, qs, nc.sync),
                           mkz(h1, zrow[(p, 1)], q, qs, nc.gpsimd),
                           nop, nop]
                    for t in range(q * 4, (q + 1) * 4):
                        nxt.append(mkop(p, t, store=(p == 1)))
                    # prepend to the NEXT quarter's schedule
                    np_, nq = (p, q + 1) if q < 3 else (p + 1, 0)
                    sched[(np_, nq)] = nxt + sched[(np_, nq)]
        while pending:
            pending.pop(0)()


def _build():
    if "/opt/trn_rl_repo" not in sys.path:
        sys.path.insert(0, "/opt/trn_rl_repo")
    from concourse import bacc, mybir
    import concourse.tile as tile

    dt = mybir.dt
    nc = bacc.Bacc("TRN2", target_bir_lowering=False, debug=False,
                   num_devices=_NCORES)
    xt = nc.dram_tensor("xt", [_DIM, _N], dt.bfloat16, kind="ExternalInput").ap()
    wq = nc.dram_tensor("wq", [_DIM, _HL], dt.bfloat16, kind="ExternalInput").ap()
    wk = nc.dram_tensor("wk", [_DIM, _HL], dt.bfloat16, kind="ExternalInput").ap()
    wv = nc.dram_tensor("wv", [_DIM, _HL], dt.bfloat16, kind="ExternalInput").ap()
    wo = nc.dram_tensor("wo", [_HL, _DIM], dt.bfloat16, kind="ExternalInput").ap()
    out = nc.dram_tensor("out", [_N, _DIM], dt.float32, kind="ExternalOutput").ap()

    with tile.TileContext(nc) as tc:
        _emit(tc, xt, wq, wk, wv, wo, out, mybir)
    nc.compile()
    return nc


def _get_nc():
    if "nc" not in _cache:
        _cache["nc"] = _build()
    return _cache["nc"]


def _shard_inputs(X, W_qkv, W_out):
    import ml_dtypes
    bf16 = ml_dtypes.bfloat16
    in_maps = []
    for c in range(_NCORES):
        b, g = c // 2, c % 2
        cols = slice(g * _HL, (g + 1) * _HL)
        in_maps.append({
            "xt": np.ascontiguousarray(X[b].T).astype(bf16),
            "wq": W_qkv[:, 0 * _DIM:][:, cols].astype(bf16),
            "wk": W_qkv[:, 1 * _DIM:][:, cols].astype(bf16),
            "wv": W_qkv[:, 2 * _DIM:][:, cols].astype(bf16),
            "wo": W_out[g * _HL:(g + 1) * _HL, :].astype(bf16),
        })
    return in_maps


def _run(inputs, trace=False):
    if "/opt/trn_rl_repo" not in sys.path:
        sys.path.insert(0, "/opt/trn_rl_repo")
    from concourse.bass_utils import run_bass_kernel_spmd

    X = np.asarray(inputs["X"], dtype=np.float32)
    W_qkv = np.asarray(inputs["W_qkv"], dtype=np.float32)
    W_out = np.asarray(inputs["W_out"], dtype=np.float32)
    b_out = np.asarray(inputs["b_out"], dtype=np.float32)

    nc = _get_nc()
    in_maps = _shard_inputs(X, W_qkv, W_out)
    res = run_bass_kernel_spmd(nc, in_maps, list(range(_NCORES)), trace=trace)

    out = np.empty((_B, _N, _DIM), dtype=np.float32)
    for b in range(_B):
        out[b] = res.results[2 * b]["out"] + res.results[2 * b + 1]["out"] + b_out
    return out, res.exec_time_ns


def kernel(**inputs) -> np.ndarray:
    out, _ = _run(inputs, trace=False)
    return out


# revision 5
# speedup vs baseline: 1.2343x; 1.2343x over previous
"""Fused self-attention (FCSelfAttention) Trainium2 Bass kernel.

Problem: X:[4,2048,512] fp32, W_qkv:[512,1536], W_out:[512,512], b_out:[512]
  qkv = X @ W_qkv ; q,k,v -> heads (B,H=8,N=2048,DH=64)
  scores[n,m] = k_n . q_m * DH**-0.5 ; softmax over m (query axis)
  out[n] = sum_e att[n,e] v[e] ; merge heads ; @ W_out + b_out

Sharding (8 cores): batch x head-group. Core c handles batch b=c//2 and
heads 4g..4g+3 where g=c%2 (data parallel over B=4, tensor parallel over
H=8 in halves). Each core computes a partial output projection for its
batch; the host sums the two partials per batch and adds b_out.

Device algorithm (per core), flash-style with scores kept transposed so
the softmax axis lands on the TensorE contraction axis:
  S^T[m,n] = sum_d QT[d,m] KT[d,n]        (m = softmax axis, on partitions)
  P^T = exp(S^T * SCALE)                   (no max subtraction; |S| < 9)
  PV:  lhsT = V_aug[e, 0:65] (col 64 = ones) -> psum[0:64]=out^T, psum[64]=Z

v2 changes over the first working kernel:
  - software pipelining: the score matmuls for block g+1 are emitted
    BEFORE block g's PV matmuls, so the (in-order) PE always has the
    next exp's input ready early and ACT never waits on PE backlog.
  - two of every 16 m-blocks compute exp on the DVE instead of ACT via
    the integer exp trick (bits of bf16 = s*16*log2e + 16250, computed
    by one tensor_scalar with int16 output); this offloads ~12.5% of
    the exp stream off the pacing Scalar engine at ~0.4% output error.
  - staged startup: only the tensors needed by quarter 0 gate the first
    exp; everything else (xt tail, remaining V/projection pieces) is
    drained through the per-block pending queue.
  - no DMAs ride the Scalar queue (they'd serialize with ACTIVATEs).
"""

import sys

import numpy as np

_B, _N, _DIM = 4, 2048, 512
_H, _DH = 8, 64
_SCALE = _DH ** -0.5
_NCORES = 8
_HPC = 4              # heads per core
_HL = _HPC * _DH      # 256 local inner dim
_TC = _N // 128       # 16 token chunks
_KC = _DIM // 128     # 4 contraction chunks for projections

# DVE integer-exp constants: bf16 bits of exp(s*SCALE) ~= s*A + B
_EXPA = 128.0 * np.log2(np.e) * _SCALE      # 16*log2(e)
_EXPB = 127.0 * 128.0 - 6.0                  # bias, c=-6 tuned offline
_DVE_ECS = (5, 11)                           # m-blocks per quarter on DVE

_cache = {}


def _emit(tc, xt, wq, wk, wv, wo, out, mybir):
    nc = tc.nc
    dt = mybir.dt
    f32, bf16, i16 = dt.float32, dt.bfloat16, dt.int16
    Exp = mybir.ActivationFunctionType.Exp
    Copy = mybir.ActivationFunctionType.Copy
    Alu = mybir.AluOpType

    from contextlib import ExitStack

    with ExitStack() as ctx:
        weights = ctx.enter_context(tc.tile_pool(name="weights", bufs=1))
        xtp = ctx.enter_context(tc.tile_pool(name="xtp", bufs=1))
        qkp = ctx.enter_context(tc.tile_pool(name="qkp", bufs=1))
        vap = ctx.enter_context(tc.tile_pool(name="vap", bufs=1))
        atp = ctx.enter_context(tc.tile_pool(name="atp", bufs=1))
        ptp = ctx.enter_context(tc.tile_pool(name="ptp", bufs=4))
        zp = ctx.enter_context(tc.tile_pool(name="zp", bufs=2))
        zdp = ctx.enter_context(tc.tile_pool(name="zdp", bufs=2, space="DRAM"))
        outp = ctx.enter_context(tc.tile_pool(name="outp", bufs=1))
        psA = ctx.enter_context(tc.tile_pool(name="psA", bufs=2, space="PSUM"))
        psOp = ctx.enter_context(tc.tile_pool(name="psO", bufs=2, space="PSUM"))
        psB = ctx.enter_context(tc.tile_pool(name="psB", bufs=2, space="PSUM"))

        # ---- input DMAs, staged ------------------------------------------
        # One large rearranged DMA per tensor (64KB chunks waste ~0.9us
        # of engine-queue time each).  Stage A gates the first exp: xt
        # token-cols 0:512 + wq/wk/wv/wo.  Stages B/C land during q0.
        xt_t = xtp.tile([128, _KC, _N], bf16, tag="xt", name="xt")
        xt_sb = [xt_t[:, kc, :] for kc in range(_KC)]
        xt_r = xt.rearrange("(kc p) n -> p kc n", p=128)
        nc.sync.dma_start(xt_t[:, :, 0:512], xt_r[:, :, 0:512])

        w_t = {}
        for name, dram, eng in (("wq", wq, nc.gpsimd), ("wk", wk, nc.sync),
                                ("wv", wv, nc.gpsimd)):
            t = weights.tile([128, _KC, _HL], bf16, tag=name, name=name)
            eng.dma_start(t, dram.rearrange("(kc p) hd -> p kc hd", p=128))
            w_t[name] = t
        wq_sb = [w_t["wq"][:, kc, :] for kc in range(_KC)]
        wk_sb = [w_t["wk"][:, kc, :] for kc in range(_KC)]
        wv_sb = [w_t["wv"][:, kc, :] for kc in range(_KC)]
        wo_t = weights.tile([128, 2, _DIM], bf16, tag="wo", name="wo")
        nc.gpsimd.dma_start(wo_t, wo.rearrange("(pr p) d -> p pr d", p=128))
        wo_sb = [wo_t[:, 0, :], wo_t[:, 1, :]]
        ones11 = weights.tile([1, 1], f32, tag="ones11", name="ones11")
        nc.vector.memset(ones11, 1.0)

        # stages B/C
        nc.sync.dma_start(xt_t[:, :, 512:1024], xt_r[:, :, 512:1024])
        nc.gpsimd.dma_start(xt_t[:, :, 1024:_N], xt_r[:, :, 1024:_N])

        # Warm the PE HAM clock with dummy matmuls while input DMAs land.
        dummy = xtp.tile([128, 512], bf16, tag="dummy", name="dummy")
        nc.vector.memset(dummy, 0.0)
        psw = psA.tile([128, 512], f32, tag="mm")
        for _ in range(8):
            nc.tensor.matmul(psw, lhsT=dummy[:, 0:128], rhs=dummy,
                             start=True, stop=True)

        # ---- qkv projections --------------------------------------------
        qt_sb = [None, None]
        kt_sb = [None, None]

        def project_qk_piece(name, wsb, lst, hc, tp, pool=None, tag="mm"):
            if lst[hc] is None:
                lst[hc] = qkp.tile([128, _N], bf16, tag=f"{name}{hc}",
                                   name=f"{name}{hc}")
            dst = lst[hc]
            ps = (pool or psA).tile([128, 512], f32, tag=tag)
            for kc in range(_KC):
                nc.tensor.matmul(
                    ps,
                    lhsT=wsb[kc][:, hc * 128:(hc + 1) * 128],
                    rhs=xt_sb[kc][:, tp * 512:(tp + 1) * 512],
                    start=(kc == 0), stop=(kc == _KC - 1),
                )
            nc.vector.tensor_copy(dst[:, tp * 512:(tp + 1) * 512], ps)

        # V augmented with a ones column: va[t][:, h, 0:64] = V, [..., 64]=1
        va_sb = []
        for t in range(_TC):
            va_sb.append(vap.tile([128, _HPC, 65], bf16, tag=f"va{t}",
                                  name=f"va{t}"))

        def v_piece(t, pool=None, tag="mm"):
            va = va_sb[t]
            nc.gpsimd.memset(va[:, :, 64:65], 1.0)
            ps = (pool or psA).tile([128, _HL], f32, tag=tag)
            for kc in range(_KC):
                nc.tensor.matmul(
                    ps,
                    lhsT=xt_sb[kc][:, t * 128:(t + 1) * 128],
                    rhs=wv_sb[kc],
                    start=(kc == 0), stop=(kc == _KC - 1),
                )
            nc.vector.tensor_copy(
                va[:, :, 0:64], ps.rearrange("p (h d) -> p h d", h=_HPC))

        # Minimum serial prefix: qt pieces 0-1, kt piece 0, va 0-3 (all
        # inside xt cols 0:512).  The rest drains through pending pops.
        project_qk_piece("qt", wq_sb, qt_sb, 0, 0, pool=psB, tag="mo")
        project_qk_piece("qt", wq_sb, qt_sb, 0, 1, pool=psB, tag="mo")
        project_qk_piece("kt", wk_sb, kt_sb, 0, 0, pool=psB, tag="mo")
        for t in range(4):
            v_piece(t, pool=psB, tag="mo")

        def mkv(t):
            return lambda: v_piece(t, pool=psB, tag="mo")

        def mkp(name, wsb, lst, hc, tp):
            return lambda: project_qk_piece(name, wsb, lst, hc, tp,
                                            pool=psB, tag="mo")

        # ---- attention state --------------------------------------------
        acc = []
        for t in range(_TC):
            acc.append(outp.tile([128, _DIM], f32, tag=f"acc{t}",
                                 name=f"acc{t}"))
        at_sb = [None, None]        # per pair, [128, N] (head rows stacked)
        zrec = [None] * _HPC

        def outproj_chunk(pair, t, store, wide=False):
            h0, h1 = 2 * pair, 2 * pair + 1
            tsl = slice(t * 128, (t + 1) * 128)
            ps0 = psB.tile([128, _DIM], f32, tag="mo")
            ps1 = (psOp if wide else psB).tile(
                [128, _DIM], f32, tag="po" if wide else "mo")
            nc.tensor.matmul(ps0, lhsT=at_sb[pair][0:64, tsl],
                             rhs=wo_sb[pair][0:64, :], start=True, stop=True)
            nc.tensor.matmul(ps1, lhsT=at_sb[pair][64:128, tsl],
                             rhs=wo_sb[pair][64:128, :], start=True, stop=True)
            if h0 == 0:
                nc.vector.tensor_scalar_mul(acc[t], ps0, zrec[h0][:, t:t + 1])
            else:
                nc.vector.scalar_tensor_tensor(
                    out=acc[t], in0=ps0, scalar=zrec[h0][:, t:t + 1],
                    in1=acc[t], op0=Alu.mult, op1=Alu.add,
                )
            nc.vector.scalar_tensor_tensor(
                out=acc[t], in0=ps1, scalar=zrec[h1][:, t:t + 1],
                in1=acc[t], op0=Alu.mult, op1=Alu.add,
            )
            if store:
                (nc.gpsimd if t % 2 else nc.sync).dma_start(
                    out[tsl, :], acc[t])

        def mkop(pair, t, store, wide=False):
            return lambda: outproj_chunk(pair, t, store, wide)

        def mkz(h, zr_row, q, qs, eng):
            def zchain():
                zd = zdp.tile([1, 512], f32, tag=f"zd{h % 2}")
                eng.dma_start(zd, zr_row[0:1, qs])
                zcol = zp.tile([128, 4], f32, tag=f"zcol{h % 2}")
                eng.dma_start(
                    zcol, zd.rearrange("o (j p) -> (o p) j", p=128))
                nc.vector.reciprocal(zrec[h][:, q * 4:(q + 1) * 4], zcol)
            return zchain

        def mkz_pe(h, zr_row, q):
            def zchain():
                pz = psB.tile([128, 4], f32, tag="mo")
                for j in range(4):
                    jj = q * 4 + j
                    nc.tensor.transpose(
                        pz[:, j:j + 1],
                        zr_row[0:1, jj * 128:(jj + 1) * 128], ones11)
                nc.vector.reciprocal(zrec[h][:, q * 4:(q + 1) * 4], pz)
            return zchain

        # ---- per-quarter pending-op schedules ---------------------------
        # quarter key (pair, q); each entry is a list of thunks popped one
        # per ec-block inside that quarter.
        nop = lambda: None  # noqa: E731
        sched = {}
        # pair0 q0: V ramp + remaining qt/kt pieces for pair 0.
        sched[(0, 0)] = [mkv(4), mkv(5), mkp("qt", wq_sb, qt_sb, 0, 2),
                         mkv(6), mkv(7), mkp("qt", wq_sb, qt_sb, 0, 3),
                         mkv(8), mkv(9), mkv(10),
                         mkp("kt", wk_sb, kt_sb, 0, 1),
                         mkv(11), mkv(12), mkv(13), mkv(14), mkv(15)]
        # later quarters get z-chains + outproj of the previous quarter
        # prepended at drain time; static part here: kt pieces for pair 0,
        # then pair 1's eight projection pieces spread over q1..q3.
        sched[(0, 1)] = [mkp("kt", wk_sb, kt_sb, 0, 2),
                         mkp("kt", wk_sb, kt_sb, 1, 0),
                         mkp("qt", wq_sb, qt_sb, 1, 0),
                         mkp("qt", wq_sb, qt_sb, 1, 1)]
        sched[(0, 2)] = [mkp("kt", wk_sb, kt_sb, 0, 3),
                         mkp("qt", wq_sb, qt_sb, 1, 2),
                         mkp("qt", wq_sb, qt_sb, 1, 3),
                         mkp("kt", wk_sb, kt_sb, 1, 1)]
        sched[(0, 3)] = [mkp("kt", wk_sb, kt_sb, 1, 2)]
        sched[(1, 0)] = [mkp("kt", wk_sb, kt_sb, 1, 3)]
        sched[(1, 1)] = []
        sched[(1, 2)] = []
        sched[(1, 3)] = []

        # ---- main loop: software-pipelined blocks ------------------------
        blocks = [(p, q, ec) for p in range(2) for q in range(4)
                  for ec in range(_TC)]

        for p in range(2):
            at_sb[p] = atp.tile([128, _N], bf16, tag=f"at{p}", name=f"at{p}")
            for h in (2 * p, 2 * p + 1):
                zrec[h] = zp.tile([128, _TC], f32, tag=f"zrec{h}",
                                  name=f"zrec{h}", bufs=1)
        zrow = {}
        for p in range(2):
            for h in range(2):
                zrow[(p, h)] = zp.tile([1, _N], f32, tag=f"zrow{p}{h}",
                                       name=f"zrow{p}{h}", bufs=1)

        def emit_scores(blk, ps):
            p, q, ec = blk
            ncol = q * 512
            nc.tensor.matmul(
                ps[:, 0:512],
                lhsT=qt_sb[p][0:64, ec * 128:(ec + 1) * 128],
                rhs=kt_sb[p][0:64, ncol:ncol + 512],
                start=True, stop=True,
            )
            nc.tensor.matmul(
                ps[:, 512:1024],
                lhsT=qt_sb[p][64:128, ec * 128:(ec + 1) * 128],
                rhs=kt_sb[p][64:128, ncol:ncol + 512],
                start=True, stop=True,
            )

        def emit_pv(blk, pt, po0, po1):
            p, q, ec = blk
            h0, h1 = 2 * p, 2 * p + 1
            nc.tensor.matmul(
                po0[0:65, :], lhsT=va_sb[ec][:, h0, :], rhs=pt[:, 0:512],
                start=(ec == 0), stop=(ec == _TC - 1),
            )
            nc.tensor.matmul(
                po1[0:65, :], lhsT=va_sb[ec][:, h1, :], rhs=pt[:, 512:1024],
                start=(ec == 0), stop=(ec == _TC - 1),
            )

        def emit_drain(p, q, po0, po1, last_q):
            # quarter drain: out^T rows -> at_sb, Z rows -> zrow
            qs = slice(q * 512, (q + 1) * 512)
            if last_q:
                nc.vector.tensor_copy(zrow[(p, 0)][:, qs], po0[64:65, :])
                nc.scalar.activation(at_sb[p][0:64, qs], po0[0:64, :], Copy)
                nc.vector.tensor_copy(zrow[(p, 1)][:, qs], po1[64:65, :])
                nc.scalar.activation(at_sb[p][64:128, qs], po1[0:64, :], Copy)
            else:
                nc.vector.tensor_copy(at_sb[p][0:64, qs], po0[0:64, :])
                nc.vector.tensor_copy(zrow[(p, 0)][:, qs], po0[64:65, :])
                nc.vector.tensor_copy(at_sb[p][64:128, qs], po1[0:64, :])
                nc.vector.tensor_copy(zrow[(p, 1)][:, qs], po1[64:65, :])

        # scores for block 0 are part of the prefix
        ps_cur = psA.tile([128, 1024], f32, tag="mm")
        emit_scores(blocks[0], ps_cur)

        # pipeline: block g emits scores(g+1), exp(g), PV(g-1), 1 pop
        po_cur = po_prev = None
        pt_prev = None
        blk_prev = None
        pending = []
        for g, blk in enumerate(blocks):
            p, q, ec = blk
            if ec == 0:
                po_cur = (psOp.tile([65, 512], f32, tag="po", name="po0"),
                          psOp.tile([65, 512], f32, tag="po", name="po1"))
                pending = sched[(p, q)] + pending

            # 1) scores for the NEXT block (PE clears ACT's next dep early)
            if g + 1 < len(blocks):
                ps_next = psA.tile([128, 1024], f32, tag="mm")
                emit_scores(blocks[g + 1], ps_next)
            # 2) exp of the current block (ACT, or DVE integer-exp)
            pt = ptp.tile([128, 1024], bf16, tag="pt")
            if ec in _DVE_ECS:
                nc.vector.tensor_scalar(
                    out=pt.bitcast(i16), in0=ps_cur,
                    scalar1=float(_EXPA), scalar2=float(_EXPB),
                    op0=Alu.mult, op1=Alu.add,
                )
            else:
                nc.scalar.activation(pt, ps_cur, Exp, scale=_SCALE)
            ps_cur = ps_next
            # 3) PV for the PREVIOUS block (its exp is long done -> no
            #    head-of-line stall on the in-order PE)
            if blk_prev is not None:
                emit_pv(blk_prev, pt_prev, *po_prev)
                if blk_prev[2] == _TC - 1:
                    emit_drain(blk_prev[0], blk_prev[1], *po_prev,
                               last_q=False)
                    pq, qq = blk_prev[0], blk_prev[1]
                    h0, h1 = 2 * pq, 2 * pq + 1
                    qs = slice(qq * 512, (qq + 1) * 512)
                    nxt = [mkz(h0, zrow[(pq, 0)], qq, qs, nc.sync),
                           mkz(h1, zrow[(pq, 1)], qq, qs, nc.gpsimd),
                           nop, nop]
                    for t in range(qq * 4, (qq + 1) * 4):
                        nxt.append(mkop(pq, t, store=(pq == 1)))
                    pending = nxt + pending
            pt_prev, blk_prev, po_prev = pt, blk, po_cur
            # 4) one deferred op
            if pending:
                pending.pop(0)()

        # final block's PV + drain + tail
        emit_pv(blk_prev, pt_prev, *po_prev)
        emit_drain(1, 3, *po_prev, last_q=True)
        mkz_pe(2, zrow[(1, 0)], 3)()
        mkz_pe(3, zrow[(1, 1)], 3)()
        for t in range(12, 16):
            outproj_chunk(1, t, store=True, wide=True)
        while pending:
            pending.pop(0)()


def _build():
    if "/opt/trn_rl_repo" not in sys.path:
        sys.path.insert(0, "/opt/trn_rl_repo")
    from concourse import bacc, mybir
    import concourse.tile as tile

    dt = mybir.dt
    nc = bacc.Bacc("TRN2", target_bir_lowering=False, debug=False,
                   num_devices=_NCORES)
    xt = nc.dram_tensor("xt", [_DIM, _N], dt.bfloat16, kind="ExternalInput").ap()
    wq = nc.dram_tensor("wq", [_DIM, _HL], dt.bfloat16, kind="ExternalInput").ap()
    wk = nc.dram_tensor("wk", [_DIM, _HL], dt.bfloat16, kind="ExternalInput").ap()
    wv = nc.dram_tensor("wv", [_DIM, _HL], dt.bfloat16, kind="ExternalInput").ap()
    wo = nc.dram_tensor("wo", [_HL, _DIM], dt.bfloat16, kind="ExternalInput").ap()
    out = nc.dram_tensor("out", [_N, _DIM], dt.float32, kind="ExternalOutput").ap()

    with tile.TileContext(nc) as tc:
        _emit(tc, xt, wq, wk, wv, wo, out, mybir)
    nc.compile()
    return nc


def _get_nc():
    if "nc" not in _cache:
        _cache["nc"] = _build()
    return _cache["nc"]


def _shard_inputs(X, W_qkv, W_out):
    import ml_dtypes
    bf16 = ml_dtypes.bfloat16
    in_maps = []
    for c in range(_NCORES):
        b, g = c // 2, c % 2
        cols = slice(g * _HL, (g + 1) * _HL)
        in_maps.append({
            "xt": np.ascontiguousarray(X[b].T).astype(bf16),
            "wq": W_qkv[:, 0 * _DIM:][:, cols].astype(bf16),
            "wk": W_qkv[:, 1 * _DIM:][:, cols].astype(bf16),
            "wv": W_qkv[:, 2 * _DIM:][:, cols].astype(bf16),
            "wo": W_out[g * _HL:(g + 1) * _HL, :].astype(bf16),
        })
    return in_maps


def _run(inputs, trace=False):
    if "/opt/trn_rl_repo" not in sys.path:
        sys.path.insert(0, "/opt/trn_rl_repo")
    from concourse.bass_utils import run_bass_kernel_spmd

    X = np.asarray(inputs["X"], dtype=np.float32)
    W_qkv = np.asarray(inputs["W_qkv"], dtype=np.float32)
    W_out = np.asarray(inputs["W_out"], dtype=np.float32)
    b_out = np.asarray(inputs["b_out"], dtype=np.float32)

    nc = _get_nc()
    in_maps = _shard_inputs(X, W_qkv, W_out)
    res = run_bass_kernel_spmd(nc, in_maps, list(range(_NCORES)), trace=trace)

    out = np.empty((_B, _N, _DIM), dtype=np.float32)
    for b in range(_B):
        out[b] = res.results[2 * b]["out"] + res.results[2 * b + 1]["out"] + b_out
    return out, res.exec_time_ns


def kernel(**inputs) -> np.ndarray:
    out, _ = _run(inputs, trace=False)
    return out


# revision 13
# speedup vs baseline: 1.2799x; 1.0370x over previous
"""Fused self-attention (FCSelfAttention) Trainium2 Bass kernel.

Problem: X:[4,2048,512] fp32, W_qkv:[512,1536], W_out:[512,512], b_out:[512]
  qkv = X @ W_qkv ; q,k,v -> heads (B,H=8,N=2048,DH=64)
  scores[n,m] = k_n . q_m * DH**-0.5 ; softmax over m (query axis)
  out[n] = sum_e att[n,e] v[e] ; merge heads ; @ W_out + b_out

Sharding (8 cores): batch x head-group. Core c handles batch b=c//2 and
heads 4g..4g+3 where g=c%2 (data parallel over B=4, tensor parallel over
H=8 in halves). Each core computes a partial output projection for its
batch; the host sums the two partials per batch and adds b_out.

Device algorithm (per core), flash-style with scores kept transposed so
the softmax axis lands on the TensorE contraction axis:
  S^T[m,n] = sum_d QT[d,m] KT[d,n]        (m = softmax axis, on partitions)
  P^T = exp(S^T * SCALE)                   (no max subtraction; |S| < 9)
  PV:  lhsT = V_aug[e, 0:65] (col 64 = ones) -> psum[0:64]=out^T, psum[64]=Z

v2 changes over the first working kernel:
  - software pipelining: the score matmuls for block g+1 are emitted
    BEFORE block g's PV matmuls, so the (in-order) PE always has the
    next exp's input ready early and ACT never waits on PE backlog.
  - two of every 16 m-blocks compute exp on the DVE instead of ACT via
    the integer exp trick (bits of bf16 = s*16*log2e + 16250, computed
    by one tensor_scalar with int16 output); this offloads ~12.5% of
    the exp stream off the pacing Scalar engine at ~0.4% output error.
  - staged startup: only the tensors needed by quarter 0 gate the first
    exp; everything else (xt tail, remaining V/projection pieces) is
    drained through the per-block pending queue.
  - no DMAs ride the Scalar queue (they'd serialize with ACTIVATEs).
"""

import sys

import numpy as np

_B, _N, _DIM = 4, 2048, 512
_H, _DH = 8, 64
_SCALE = _DH ** -0.5
_NCORES = 8
_HPC = 4              # heads per core
_HL = _HPC * _DH      # 256 local inner dim
_TC = _N // 128       # 16 token chunks
_KC = _DIM // 128     # 4 contraction chunks for projections

# DVE integer-exp constants: bf16 bits of exp(s*SCALE) ~= s*A + B
_EXPA = 128.0 * np.log2(np.e) * _SCALE      # 16*log2(e)
_EXPB = 127.0 * 128.0 - 6.0                  # bias, c=-6 tuned offline
_DVE_ECS = (5, 11)                           # m-blocks per quarter on DVE

_cache = {}


def _emit(tc, xt, wq, wk, wv, wo, out, mybir):
    nc = tc.nc
    dt = mybir.dt
    f32, bf16, i16 = dt.float32, dt.bfloat16, dt.int16
    Exp = mybir.ActivationFunctionType.Exp
    Copy = mybir.ActivationFunctionType.Copy
    Alu = mybir.AluOpType

    from contextlib import ExitStack

    with ExitStack() as ctx:
        weights = ctx.enter_context(tc.tile_pool(name="weights", bufs=1))
        xtp = ctx.enter_context(tc.tile_pool(name="xtp", bufs=1))
        qkp = ctx.enter_context(tc.tile_pool(name="qkp", bufs=1))
        vap = ctx.enter_context(tc.tile_pool(name="vap", bufs=1))
        atp = ctx.enter_context(tc.tile_pool(name="atp", bufs=1))
        ptp = ctx.enter_context(tc.tile_pool(name="ptp", bufs=4))
        zp = ctx.enter_context(tc.tile_pool(name="zp", bufs=2))
        zdp = ctx.enter_context(tc.tile_pool(name="zdp", bufs=2, space="DRAM"))
        outp = ctx.enter_context(tc.tile_pool(name="outp", bufs=1))
        psA = ctx.enter_context(tc.tile_pool(name="psA", bufs=2, space="PSUM"))
        psOp = ctx.enter_context(tc.tile_pool(name="psO", bufs=2, space="PSUM"))
        psB = ctx.enter_context(tc.tile_pool(name="psB", bufs=2, space="PSUM"))

        # ---- input DMAs, staged ------------------------------------------
        # Contiguous per-chunk DMAs (rearranged single DMAs generate 512B
        # scattered descriptors, ~5x slower), spread across the sync /
        # gpsimd / vector queues.  Stage A (xt cols 0:512 + all weights)
        # gates the first exp; stages B/C land during quarter 0.
        ones11 = weights.tile([1, 1], f32, tag="ones11", name="ones11")
        nc.vector.memset(ones11, 1.0)
        dummy = xtp.tile([128, 512], bf16, tag="dummy", name="dummy")
        nc.vector.memset(dummy, 0.0)
        xt_sb = []
        for kc in range(_KC):
            t = xtp.tile([128, _N], bf16, tag=f"xt{kc}", name=f"xt{kc}")
            xt_sb.append(t)
        for kc in range(_KC):
            (nc.sync if kc < 2 else nc.gpsimd).dma_start(
                xt_sb[kc][:, 0:512], xt[kc * 128:(kc + 1) * 128, 0:512])

        wq_sb, wk_sb, wv_sb = [], [], []
        for name, dram, lst, eng in (
                ("wq", wq, wq_sb, nc.gpsimd), ("wk", wk, wk_sb, nc.sync),
                ("wv", wv, wv_sb, nc.scalar)):
            for kc in range(_KC):
                t = weights.tile([128, _HL], bf16, tag=f"{name}{kc}",
                                 name=f"{name}{kc}")
                eng.dma_start(t, dram[kc * 128:(kc + 1) * 128, :])
                lst.append(t)
        wo_sb = []
        for pair in range(2):
            t = weights.tile([128, _DIM], bf16, tag=f"wo{pair}",
                             name=f"wo{pair}")
            nc.scalar.dma_start(t, wo[pair * 128:(pair + 1) * 128, :])
            wo_sb.append(t)
        # stages B/C
        for lo, hi in ((512, 1024), (1024, _N)):
            for kc in range(_KC):
                (nc.sync if kc < 2 else nc.gpsimd).dma_start(
                    xt_sb[kc][:, lo:hi], xt[kc * 128:(kc + 1) * 128, lo:hi])

        # Warm the PE HAM clock with dummy matmuls while input DMAs land.
        psw = psA.tile([128, 512], f32, tag="mm")
        for _ in range(6):
            nc.tensor.matmul(psw, lhsT=dummy[:, 0:128], rhs=dummy,
                             start=True, stop=True)

        # ---- qkv projections --------------------------------------------
        qt_sb = [None, None]
        kt_sb = [None, None]

        def project_qk_piece(name, wsb, lst, hc, tp, pool=None, tag="mm"):
            if lst[hc] is None:
                lst[hc] = qkp.tile([128, _N], bf16, tag=f"{name}{hc}",
                                   name=f"{name}{hc}")
            dst = lst[hc]
            ps = (pool or psA).tile([128, 512], f32, tag=tag)
            for kc in range(_KC):
                nc.tensor.matmul(
                    ps,
                    lhsT=wsb[kc][:, hc * 128:(hc + 1) * 128],
                    rhs=xt_sb[kc][:, tp * 512:(tp + 1) * 512],
                    start=(kc == 0), stop=(kc == _KC - 1),
                )
            nc.vector.tensor_copy(dst[:, tp * 512:(tp + 1) * 512], ps)

        # V augmented with a ones column: va[t][:, h, 0:64] = V, [..., 64]=1
        va_sb = []
        for t in range(_TC):
            va_sb.append(vap.tile([128, _HPC, 65], bf16, tag=f"va{t}",
                                  name=f"va{t}"))

        def v_piece(t, pool=None, tag="mm"):
            va = va_sb[t]
            nc.gpsimd.memset(va[:, :, 64:65], 1.0)
            ps = (pool or psA).tile([128, _HL], f32, tag=tag)
            for kc in range(_KC):
                nc.tensor.matmul(
                    ps,
                    lhsT=xt_sb[kc][:, t * 128:(t + 1) * 128],
                    rhs=wv_sb[kc],
                    start=(kc == 0), stop=(kc == _KC - 1),
                )
            nc.vector.tensor_copy(
                va[:, :, 0:64], ps.rearrange("p (h d) -> p h d", h=_HPC))

        # Minimum serial prefix: qt pieces 0-1, kt piece 0, va 0-2 (all
        # inside xt cols 0:512).  The rest drains through pending pops.
        project_qk_piece("qt", wq_sb, qt_sb, 0, 0, pool=psB, tag="mo")
        project_qk_piece("qt", wq_sb, qt_sb, 0, 1, pool=psB, tag="mo")
        project_qk_piece("kt", wk_sb, kt_sb, 0, 0, pool=psB, tag="mo")
        for t in range(3):
            v_piece(t, pool=psB, tag="mo")

        def mkv(t):
            return lambda: v_piece(t, pool=psB, tag="mo")

        def mkp(name, wsb, lst, hc, tp):
            return lambda: project_qk_piece(name, wsb, lst, hc, tp,
                                            pool=psB, tag="mo")

        # ---- attention state --------------------------------------------
        acc = []
        for t in range(_TC):
            acc.append(outp.tile([128, _DIM], f32, tag=f"acc{t}",
                                 name=f"acc{t}"))
        at_sb = [None, None]        # per pair, [128, N] (head rows stacked)
        zrec = [None] * _HPC

        def outproj_chunk(pair, t, store, wide=False):
            h0, h1 = 2 * pair, 2 * pair + 1
            tsl = slice(t * 128, (t + 1) * 128)
            ps0 = psB.tile([128, _DIM], f32, tag="mo")
            ps1 = (psOp if wide else psB).tile(
                [128, _DIM], f32, tag="po" if wide else "mo")
            nc.tensor.matmul(ps0, lhsT=at_sb[pair][0:64, tsl],
                             rhs=wo_sb[pair][0:64, :], start=True, stop=True)
            nc.tensor.matmul(ps1, lhsT=at_sb[pair][64:128, tsl],
                             rhs=wo_sb[pair][64:128, :], start=True, stop=True)
            if h0 == 0:
                nc.vector.tensor_scalar_mul(acc[t], ps0, zrec[h0][:, t:t + 1])
            else:
                nc.vector.scalar_tensor_tensor(
                    out=acc[t], in0=ps0, scalar=zrec[h0][:, t:t + 1],
                    in1=acc[t], op0=Alu.mult, op1=Alu.add,
                )
            nc.vector.scalar_tensor_tensor(
                out=acc[t], in0=ps1, scalar=zrec[h1][:, t:t + 1],
                in1=acc[t], op0=Alu.mult, op1=Alu.add,
            )
            if store:
                (nc.gpsimd if t % 2 else nc.sync).dma_start(
                    out[tsl, :], acc[t])

        def mkop(pair, t, store, wide=False):
            return lambda: outproj_chunk(pair, t, store, wide)

        def mkz(h, zr_row, q, qs, eng):
            def zchain():
                zd = zdp.tile([1, 512], f32, tag=f"zd{h % 2}")
                eng.dma_start(zd, zr_row[0:1, qs])
                zcol = zp.tile([128, 4], f32, tag=f"zcol{h % 2}")
                eng.dma_start(
                    zcol, zd.rearrange("o (j p) -> (o p) j", p=128))
                nc.vector.reciprocal(zrec[h][:, q * 4:(q + 1) * 4], zcol)
            return zchain

        def mkz_pe(h, zr_row, q):
            def zchain():
                pz = psB.tile([128, 4], f32, tag="mo")
                for j in range(4):
                    jj = q * 4 + j
                    nc.tensor.transpose(
                        pz[:, j:j + 1],
                        zr_row[0:1, jj * 128:(jj + 1) * 128], ones11)
                nc.vector.reciprocal(zrec[h][:, q * 4:(q + 1) * 4], pz)
            return zchain

        # ---- per-quarter pending-op schedules ---------------------------
        # quarter key (pair, q); each entry is a list of thunks popped one
        # per ec-block inside that quarter.
        nop = lambda: None  # noqa: E731
        sched = {}
        # pair0 q0: V ramp + remaining qt/kt pieces for pair 0.
        sched[(0, 0)] = [mkv(3), mkv(4), mkp("qt", wq_sb, qt_sb, 0, 2),
                         mkv(5), mkv(6), mkp("qt", wq_sb, qt_sb, 0, 3),
                         mkv(7), mkv(8), mkv(9),
                         mkp("kt", wk_sb, kt_sb, 0, 1),
                         mkv(10), mkv(11), mkv(12), mkv(13), mkv(14),
                         mkv(15)]
        # later quarters: the drain prepends z-chains + outproj of the
        # previous quarter, interleaved with these static pieces so no
        # two heavy PE pops land in adjacent blocks.
        sched[(0, 1)] = [mkp("kt", wk_sb, kt_sb, 0, 2),
                         mkp("kt", wk_sb, kt_sb, 1, 0),
                         mkp("qt", wq_sb, qt_sb, 1, 0),
                         mkp("qt", wq_sb, qt_sb, 1, 1)]
        sched[(0, 2)] = [mkp("kt", wk_sb, kt_sb, 0, 3),
                         mkp("qt", wq_sb, qt_sb, 1, 2),
                         mkp("qt", wq_sb, qt_sb, 1, 3),
                         mkp("kt", wk_sb, kt_sb, 1, 1)]
        sched[(0, 3)] = [mkp("kt", wk_sb, kt_sb, 1, 2)]
        sched[(1, 0)] = [mkp("kt", wk_sb, kt_sb, 1, 3)]
        sched[(1, 1)] = []
        sched[(1, 2)] = []
        sched[(1, 3)] = []

        def merge_quarter(zops, ops, statics):
            # [z0, z1, s0, op0, s1, op1, ...] - pieces spaced 2 apart,
            # first op ~3 blocks after its z-chain is issued.
            lst = list(zops)
            a, b = list(statics), list(ops)
            while a or b:
                if a:
                    lst.append(a.pop(0))
                if b:
                    lst.append(b.pop(0))
            return lst

        # ---- main loop: software-pipelined blocks ------------------------
        blocks = [(p, q, ec) for p in range(2) for q in range(4)
                  for ec in range(_TC)]

        for p in range(2):
            at_sb[p] = atp.tile([128, _N], bf16, tag=f"at{p}", name=f"at{p}")
            for h in (2 * p, 2 * p + 1):
                zrec[h] = zp.tile([128, _TC], f32, tag=f"zrec{h}",
                                  name=f"zrec{h}", bufs=1)
        zrow = {}
        for p in range(2):
            for h in range(2):
                zrow[(p, h)] = zp.tile([1, _N], f32, tag=f"zrow{p}{h}",
                                       name=f"zrow{p}{h}", bufs=1)

        def emit_scores(blk, ps):
            p, q, ec = blk
            ncol = q * 512
            nc.tensor.matmul(
                ps[:, 0:512],
                lhsT=qt_sb[p][0:64, ec * 128:(ec + 1) * 128],
                rhs=kt_sb[p][0:64, ncol:ncol + 512],
                start=True, stop=True,
            )
            nc.tensor.matmul(
                ps[:, 512:1024],
                lhsT=qt_sb[p][64:128, ec * 128:(ec + 1) * 128],
                rhs=kt_sb[p][64:128, ncol:ncol + 512],
                start=True, stop=True,
            )

        def emit_pv(blk, pt, po0, po1):
            p, q, ec = blk
            h0, h1 = 2 * p, 2 * p + 1
            nc.tensor.matmul(
                po0[0:65, :], lhsT=va_sb[ec][:, h0, :], rhs=pt[:, 0:512],
                start=(ec == 0), stop=(ec == _TC - 1),
            )
            nc.tensor.matmul(
                po1[0:65, :], lhsT=va_sb[ec][:, h1, :], rhs=pt[:, 512:1024],
                start=(ec == 0), stop=(ec == _TC - 1),
            )

        def emit_drain(p, q, po0, po1, last_q):
            # quarter drain: out^T rows -> at_sb, Z rows -> zrow
            qs = slice(q * 512, (q + 1) * 512)
            if last_q:
                nc.vector.tensor_copy(zrow[(p, 0)][:, qs], po0[64:65, :])
                nc.scalar.activation(at_sb[p][0:64, qs], po0[0:64, :], Copy)
                nc.vector.tensor_copy(zrow[(p, 1)][:, qs], po1[64:65, :])
                nc.scalar.activation(at_sb[p][64:128, qs], po1[0:64, :], Copy)
            else:
                nc.vector.tensor_copy(at_sb[p][0:64, qs], po0[0:64, :])
                nc.vector.tensor_copy(zrow[(p, 0)][:, qs], po0[64:65, :])
                nc.vector.tensor_copy(at_sb[p][64:128, qs], po1[0:64, :])
                nc.vector.tensor_copy(zrow[(p, 1)][:, qs], po1[64:65, :])

        # scores for block 0 are part of the prefix
        ps_cur = psA.tile([128, 1024], f32, tag="mm")
        emit_scores(blocks[0], ps_cur)

        # pipeline: block g emits scores(g+1), exp(g), PV(g-1), 1 pop
        po_cur = po_prev = None
        pt_prev = None
        blk_prev = None
        pt_early = None
        pending = list(sched[(0, 0)])
        for g, blk in enumerate(blocks):
            p, q, ec = blk
            if ec == 0:
                po_cur = (psOp.tile([65, 512], f32, tag="po", name="po0"),
                          psOp.tile([65, 512], f32, tag="po", name="po1"))

            # 1) scores for the NEXT block (PE clears ACT's next dep early)
            if g + 1 < len(blocks):
                ps_next = psA.tile([128, 1024], f32, tag="mm")
                emit_scores(blocks[g + 1], ps_next)
            # 2) exp of the current block.  DVE integer-exp blocks were
            #    already emitted one block early (queue-latency headroom).
            if pt_early is not None:
                pt = pt_early
                pt_early = None
            else:
                pt = ptp.tile([128, 1024], bf16, tag="pt")
                nc.scalar.activation(pt, ps_cur, Exp, scale=_SCALE)
            if g + 1 < len(blocks) and blocks[g + 1][2] in _DVE_ECS:
                pt_early = ptp.tile([128, 1024], bf16, tag="pt",
                                    name="pt_early")
                nc.vector.tensor_scalar(
                    out=pt_early.bitcast(i16), in0=ps_next,
                    scalar1=float(_EXPA), scalar2=float(_EXPB),
                    op0=Alu.mult, op1=Alu.add,
                )
            ps_cur = ps_next
            # 3) PV for the PREVIOUS block (its exp is long done -> no
            #    head-of-line stall on the in-order PE)
            if blk_prev is not None:
                emit_pv(blk_prev, pt_prev, *po_prev)
                if blk_prev[2] == _TC - 1:
                    emit_drain(blk_prev[0], blk_prev[1], *po_prev,
                               last_q=False)
                    pq, qq = blk_prev[0], blk_prev[1]
                    h0, h1 = 2 * pq, 2 * pq + 1
                    qs = slice(qq * 512, (qq + 1) * 512)
                    zops = [mkz(h0, zrow[(pq, 0)], qq, qs, nc.sync),
                            mkz(h1, zrow[(pq, 1)], qq, qs, nc.gpsimd)]
                    ops = [mkop(pq, t, store=(pq == 1))
                           for t in range(qq * 4, (qq + 1) * 4)]
                    pending = merge_quarter(zops, ops, sched[(p, q)]) \
                        + pending
            pt_prev, blk_prev, po_prev = pt, blk, po_cur
            # 4) one deferred op
            if pending:
                pending.pop(0)()

        # final block's PV + drain + tail
        emit_pv(blk_prev, pt_prev, *po_prev)
        emit_drain(1, 3, *po_prev, last_q=True)
        mkz_pe(2, zrow[(1, 0)], 3)()
        mkz_pe(3, zrow[(1, 1)], 3)()
        for t in range(12, 16):
            outproj_chunk(1, t, store=True, wide=True)
        while pending:
            pending.pop(0)()


def _build():
    if "/opt/trn_rl_repo" not in sys.path:
        sys.path.insert(0, "/opt/trn_rl_repo")
    from concourse import bacc, mybir
    import concourse.tile as tile

    dt = mybir.dt
    nc = bacc.Bacc("TRN2", target_bir_lowering=False, debug=False,
                   num_devices=_NCORES)
    xt = nc.dram_tensor("xt", [_DIM, _N], dt.bfloat16, kind="ExternalInput").ap()
    wq = nc.dram_tensor("wq", [_DIM, _HL], dt.bfloat16, kind="ExternalInput").ap()
    wk = nc.dram_tensor("wk", [_DIM, _HL], dt.bfloat16, kind="ExternalInput").ap()
    wv = nc.dram_tensor("wv", [_DIM, _HL], dt.bfloat16, kind="ExternalInput").ap()
    wo = nc.dram_tensor("wo", [_HL, _DIM], dt.bfloat16, kind="ExternalInput").ap()
    out = nc.dram_tensor("out", [_N, _DIM], dt.float32, kind="ExternalOutput").ap()

    with tile.TileContext(nc) as tc:
        _emit(tc, xt, wq, wk, wv, wo, out, mybir)
    nc.compile()
    return nc


def _get_nc():
    if "nc" not in _cache:
        _cache["nc"] = _build()
    return _cache["nc"]


def _shard_inputs(X, W_qkv, W_out):
    import ml_dtypes
    bf16 = ml_dtypes.bfloat16
    in_maps = []
    for c in range(_NCORES):
        b, g = c // 2, c % 2
        cols = slice(g * _HL, (g + 1) * _HL)
        in_maps.append({
            "xt": np.ascontiguousarray(X[b].T).astype(bf16),
            "wq": W_qkv[:, 0 * _DIM:][:, cols].astype(bf16),
            "wk": W_qkv[:, 1 * _DIM:][:, cols].astype(bf16),
            "wv": W_qkv[:, 2 * _DIM:][:, cols].astype(bf16),
            "wo": W_out[g * _HL:(g + 1) * _HL, :].astype(bf16),
        })
    return in_maps


def _run(inputs, trace=False):
    if "/opt/trn_rl_repo" not in sys.path:
        sys.path.insert(0, "/opt/trn_rl_repo")
    from concourse.bass_utils import run_bass_kernel_spmd

    X = np.asarray(inputs["X"], dtype=np.float32)
    W_qkv = np.asarray(inputs["W_qkv"], dtype=np.float32)
    W_out = np.asarray(inputs["W_out"], dtype=np.float32)
    b_out = np.asarray(inputs["b_out"], dtype=np.float32)

    nc = _get_nc()
    in_maps = _shard_inputs(X, W_qkv, W_out)
    res = run_bass_kernel_spmd(nc, in_maps, list(range(_NCORES)), trace=trace)

    out = np.empty((_B, _N, _DIM), dtype=np.float32)
    for b in range(_B):
        out[b] = res.results[2 * b]["out"] + res.results[2 * b + 1]["out"] + b_out
    return out, res.exec_time_ns


def kernel(**inputs) -> np.ndarray:
    out, _ = _run(inputs, trace=False)
    return out
